# revision 9
# baseline (speedup 1.0000x reference)
"""Trainium2 Bass kernel: per-batch cosine-distance matrix.

out[b] = 1 - metric[b] @ metric[b].T   where metric = x / ||x||_2 (last dim)
x: [32, 1024, 768] f32  ->  out: [32, 1024, 1024] f32

Sharding: data-parallel over batch. 8 cores x 4 batches each; no
cross-core communication.

Strategy (fp8 DoubleRow):
  Host prep: cast x to fp8 e4m3 and transpose each batch to xT8 [C, T]
  (layout prep only - all math runs on device). Device, per batch:
    1. DMA xT8 -> SBUF [128, (k, t)] fp8 (6 k-chunks of C).
    2. PE: 8 diagonal blocks of the RAW Gram via fp8 DoubleRow matmuls
       (K-pairs of 128 partitions -> 0.5 cyc/row).
    3. Diag extract (Pool scalar_tensor_tensor vs identity, accum_out)
       -> ss[t] = ||x8_t||^2 in f32; DVE reciprocal + ACT sqrt ->
       rinv = SCL/||x8_t||.
    4. PE f32 transpose [128,8] -> [8,128]; ACT copy -> bf16 row;
       DMA flatten to DRAM row [T]; DMA partition-broadcast back ->
       RI [128, T] bf16.
    5. met8 = x8 * RI (fp8), split DVE/Pool.
    6. Full Gram on met8: 8 row-blocks x 3 DoubleRow matmuls x 2 halves
       into [128, 1024] f32 psum (2 banks).
    7. Evict: out = 1 - psum/SCL^2 -> f16, split ACT/DVE/Pool.
    8. DMA out f16 rows, split across SP + ACT hwdge queues.
  Host post: concat + upcast f16 -> f32.

3-deep software pipeline, iteration i:
  load(i+3) | met8(i+1) | gram+evict rows(i), with diag(i+2) after row 0
  and the rinv transpose/broadcast chain(i+2) after row 2 - so RI(b) is
  always a full iteration old when met8(b) runs, hiding the DMA bounce.
"""

import sys
import time
from contextlib import ExitStack

_TRN_REPO = "/opt/trn_rl_repo"
if _TRN_REPO not in sys.path:
    sys.path.insert(0, _TRN_REPO)

import numpy as np
import ml_dtypes

import concourse.bacc as bacc
import concourse.mybir as mybir
import concourse.tile as tile
from concourse.bass_utils import run_bass_kernel_spmd
from concourse.masks import make_identity

B, T, C = 32, 1024, 768
N_CORES = 8
BPC = B // N_CORES   # batches per core
KC = C // 128        # 6 k-chunks
KP = KC // 2         # 3 k-pairs (DoubleRow)
TT = T // 128        # 8 row blocks
SCL = 8.0            # fp8 range scale for met8
F32 = mybir.dt.float32
F16 = mybir.dt.float16
BF16 = mybir.dt.bfloat16
F8 = mybir.dt.float8e4
AF = mybir.ActivationFunctionType
ALU = mybir.AluOpType
DR = mybir.MatmulPerfMode.DoubleRow

# engine split knobs (tuned from traces)
# NOTE: GpSimd/Pool cannot access PSUM (BIR verifier) and runs tensor ops
# at ~0.42x roofline (software Q7) - so Pool only gets a column-slice of
# the met8 scaling; PSUM evictions go to ACT; extracts to DVE.
MET8_DVE_CHUNKS = [0, 1, 2, 3]   # met8 k-chunks on DVE (flat 2D ops)
MET8_SPLIT4 = 512                # chunk 4: [0:split] DVE, rest Pool
OUT_Q = ["s", "a", "s", "a", "s", "a", "s", "a"]   # out DMA queue per row


def build():
    nc = bacc.Bacc("TRN2", target_bir_lowering=False, debug=False,
                   num_devices=N_CORES)
    xT8 = nc.dram_tensor("xT8", [BPC, C, T], F8, kind="ExternalInput").ap()
    out = nc.dram_tensor("out", [BPC, T, T], F16, kind="ExternalOutput").ap()
    rowsc = nc.dram_tensor("rowsc", [BPC, T], BF16, kind="Internal").ap()

    with tile.TileContext(nc) as tc, ExitStack() as ctx:
        x_pool = ctx.enter_context(tc.tile_pool(name="x", bufs=4))
        m_pool = ctx.enter_context(tc.tile_pool(name="m", bufs=2))
        s_pool = ctx.enter_context(tc.tile_pool(name="s", bufs=2))
        ri_pool = ctx.enter_context(tc.tile_pool(name="ri", bufs=2))
        ob_pool = ctx.enter_context(tc.tile_pool(name="ob", bufs=8))
        c_pool = ctx.enter_context(tc.tile_pool(name="c", bufs=1))
        psd_pool = ctx.enter_context(
            tc.tile_pool(name="psd", bufs=1, space="PSUM"))
        psT_pool = ctx.enter_context(
            tc.tile_pool(name="psT", bufs=1, space="PSUM"))
        psg_pool = ctx.enter_context(
            tc.tile_pool(name="psg", bufs=2, space="PSUM"))

        identf = c_pool.tile([128, 128], F32)
        make_identity(nc, identf[:])
        # tiled identity [128, TT*128] bf16: one identity block per row-block
        TI = c_pool.tile([128, TT * 128], BF16, tag="TI")
        for m in range(TT):
            make_identity(nc, TI[:, m * 128:(m + 1) * 128])

        # warm the ACT Sqrt table while the first DMA flies
        warm = c_pool.tile([128, 1], F32, tag="warm")
        nc.vector.memset(warm[:], 1.0)
        warm2 = c_pool.tile([128, 1], F32, tag="warm2")
        nc.scalar.sqrt(warm2[:], warm[:])

        x83s, met83s, RIs = {}, {}, {}

        def emit_load(b):
            x8 = x_pool.tile([128, KC * T], F8, tag="x8", name=f"x8_{b}")
            nc.sync.dma_start(x8[:].rearrange("p (k t) -> p k t", k=KC),
                              xT8[b].rearrange("(k p) t -> p k t", p=128))
            x83s[b] = x8[:].rearrange("p (k t) -> p k t", k=KC)

        def emit_diag(b):
            # raw-gram diagonal blocks -> ss -> rinv (f32 [128, TT])
            x83 = x83s[b]
            pd = psd_pool.tile([128, TT * 128], F32, tag="pd",
                               name=f"pd_{b}")
            rv = s_pool.tile([128, TT], F32, tag="rv", name=f"rv_{b}")
            pdsb = s_pool.tile([128, TT * 128], F32, tag="pdsb",
                               name=f"pdsb_{b}")
            for m in range(TT):
                sl = slice(m * 128, (m + 1) * 128)
                for j in range(KP):
                    nc.tensor.matmul(pd[:, sl], x83[:, 2 * j:2 * j + 2, sl],
                                     x83[:, 2 * j:2 * j + 2, sl],
                                     start=(j == 0), stop=(j == KP - 1),
                                     perf_mode=DR)
                if m % 2 == 1:
                    # copy a pair of finished diag blocks off PSUM (ACT),
                    # Pool masks them against the tiled identity
                    psl = slice((m - 1) * 128, (m + 1) * 128)
                    nc.scalar.activation(pdsb[:, psl], pd[:, psl], AF.Copy,
                                         bias=0.0, scale=1.0)
                    nc.gpsimd.tensor_tensor(pdsb[:, psl], pdsb[:, psl],
                                            TI[:, psl], ALU.mult)
            # one 3D reduce collapses all 8 masked blocks -> rv [128, TT]
            nc.vector.tensor_reduce(
                rv[:], pdsb[:].rearrange("p (m q) -> p m q", m=TT),
                mybir.AxisListType.X, ALU.add)
            rr = s_pool.tile([128, TT], F32, tag="rr", name=f"rr_{b}")
            nc.vector.reciprocal(rr[:], rv[:])
            rinvv = s_pool.tile([128, TT], F32, tag="rinvv", name=f"riv_{b}")
            nc.scalar.activation(rinvv[:], rr[:], AF.Sqrt, bias=0.0,
                                 scale=SCL * SCL)
            return rinvv

        def emit_ri_chain(b, rinvv):
            # rinv [128, TT] -> RI [128, T] bf16 (broadcast along partitions)
            rvT = psT_pool.tile([TT, 128], F32, tag="rvT", name=f"rvT_{b}")
            nc.tensor.transpose(rvT[:], rinvv[:], identf[:])
            row8 = s_pool.tile([TT, 128], BF16, tag="row8", name=f"row8_{b}")
            nc.vector.tensor_copy(row8[:], rvT[:])
            nc.scalar.dma_start(rowsc[b], row8[:])
            RI = ri_pool.tile([128, T], BF16, tag="RI", name=f"RI_{b}")
            nc.scalar.dma_start(
                RI[:], rowsc[b].unsqueeze(0).to_broadcast((128, T)))
            RIs[b] = RI

        def emit_met8(b):
            x83, RI = x83s[b], RIs[b]
            met8 = m_pool.tile([128, KC * T], F8, tag="met8",
                               name=f"met8_{b}")
            met83 = met8[:].rearrange("p (k t) -> p k t", k=KC)
            sp = MET8_SPLIT4
            for k in range(KC):
                if k in MET8_DVE_CHUNKS:
                    nc.vector.tensor_tensor(met83[:, k, :], x83[:, k, :],
                                            RI[:], ALU.mult)
                elif k == 4:
                    nc.vector.tensor_tensor(met83[:, k, :sp], x83[:, k, :sp],
                                            RI[:, :sp], ALU.mult)
                    nc.gpsimd.tensor_tensor(met83[:, k, sp:], x83[:, k, sp:],
                                            RI[:, sp:], ALU.mult)
                else:
                    nc.gpsimd.tensor_tensor(met83[:, k, :], x83[:, k, :],
                                            RI[:], ALU.mult)
            met83s[b] = met83

        def emit_gram_row(b, m):
            # upper triangle only: row m covers s in [m*128, T); the host
            # mirrors the symmetric lower half.
            met83 = met83s[b]
            n0 = m * 128
            W = T - n0
            pg = psg_pool.tile([128, T], F32, tag="pg", name=f"pg_{b}_{m}")
            sl = slice(n0, n0 + 128)
            off = 0
            while off < W:
                w = min(512, W - off)
                hs = slice(n0 + off, n0 + off + w)
                for j in range(KP):
                    nc.tensor.matmul(pg[:, off:off + w],
                                     met83[:, 2 * j:2 * j + 2, sl],
                                     met83[:, 2 * j:2 * j + 2, hs],
                                     start=(j == 0), stop=(j == KP - 1),
                                     perf_mode=DR)
                off += w
            ob = ob_pool.tile([128, T], F16, tag="ob", name=f"ob_{b}_{m}")
            nscl = -1.0 / (SCL * SCL)
            nc.scalar.activation(ob[:, :W], pg[:, :W], AF.Copy, bias=1.0,
                                 scale=nscl)
            eng = nc.sync if OUT_Q[m] == "s" else nc.scalar
            eng.dma_start(out[b, n0:n0 + 128, n0:], ob[:, :W])

        # ---- 3-deep software pipeline ----
        rinvvs = {}
        for i in range(-3, BPC):
            if 0 <= i + 3 < BPC:
                emit_load(i + 3)
            if 0 <= i + 1 < BPC and (i + 1) in RIs:
                emit_met8(i + 1)
            if i >= 0:
                for m in range(TT):
                    emit_gram_row(i, m)
                    if m == 0 and i + 2 < BPC:
                        rinvvs[i + 2] = emit_diag(i + 2)
                    elif m == 2 and i + 2 < BPC:
                        emit_ri_chain(i + 2, rinvvs[i + 2])
            else:
                b = i + 2
                if 0 <= b < BPC:
                    rinvvs[b] = emit_diag(b)
                    emit_ri_chain(b, rinvvs[b])

    nc.compile()
    return nc


_MIRROR_MASK = None


def host_post(upper_f16):
    """Mirror the upper triangle onto the (unwritten) lower half, f32."""
    global _MIRROR_MASK
    if _MIRROR_MASK is None:
        idx = np.arange(T)
        _MIRROR_MASK = (idx[None, :] >= idx[:, None])[None]  # j >= i
    u = upper_f16.astype(np.float32)
    return np.where(_MIRROR_MASK, u, u.transpose(0, 2, 1))


def host_prep(x):
    x = np.asarray(x)
    x8 = x.astype(ml_dtypes.float8_e4m3)               # [B, T, C]
    xT8 = np.ascontiguousarray(x8.transpose(0, 2, 1))  # [B, C, T]
    return xT8


def run(x, trace=False):
    nc = build()
    xT8 = host_prep(x)
    in_maps = [{"xT8": xT8[i * BPC:(i + 1) * BPC]} for i in range(N_CORES)]
    last_err = None
    for _attempt in range(3):
        try:
            res = run_bass_kernel_spmd(nc, in_maps, list(range(N_CORES)),
                                       trace=trace)
            break
        except Exception as e:  # transient device wedge: retry
            last_err = e
            time.sleep(2.0)
    else:
        raise last_err
    out = np.concatenate([host_post(res.results[i]["out"])
                          for i in range(N_CORES)], axis=0)
    return out, res


def kernel(x):
    out, _ = run(x, trace=False)
    return out
